# revision 3
# baseline (speedup 1.0000x reference)
"""Distributed KNN (analogy-based estimation) kernel for 8 TRN2 NeuronCores.

Strategy (scan-then-refine):
  - Shard the train set (N=65536) across 8 cores (8192 rows each); replicate
    the 2048 queries.  All tensors fit in SBUF, so HBM traffic is just the
    ~3MB/core input load.  No collectives - the merge happens on the host.
  - Device scan: fp8(e4m3) DoubleRow matmuls (K=256 in one instruction)
    compute s = scale * (x_hat . t) into PSUM f32.  The true distance's norm
    term only perturbs candidate ranking by O(1) while top-of-65536 gaps are
    O(10), so the cross term alone selects candidate cells safely.
  - Evacuation is split three ways: 1 of 4 PSUM tiles goes straight to
    VectorE (reduce_max over 32-candidate cells); the other 3 go ScalarE
    (fused relu(s - T), made row-comparable by host query normalization) ->
    GpSimd (ADD-fold of tile halves) -> VectorE (small sum-reduce),
    producing a 256-cell statistic vector per (row, core) that is DMA'd out
    raw (top-k selection on host beats on-device max/max_index).
  - Host: top-16 cells per (row, core) by statistic, expand to ~4k candidate
    indices/row as contiguous 16-row blocks, coarse f32 distance pass
    narrows to 8 finalists, exact float64 pass ranks them with the
    reference's tie-breaking, then the label gather / faithful [B,k]->[k,B]
    reshape / integer-mean / one-hot epilogue in exact integer arithmetic.
"""

from contextlib import ExitStack

import numpy as np
import ml_dtypes

import concourse.bass as bass
import concourse.mybir as mybir
import concourse.tile as tile
from concourse import bacc
from concourse.bass_utils import run_bass_kernel_spmd

N_CORES = 8
B = 2048          # queries
N_TRAIN = 65536   # train rows
F = 256           # features
NSHARD = N_TRAIN // N_CORES   # 8192 train rows per core

Q_TILE = 128
N_QT = B // Q_TILE            # 16 query tiles
CHUNK_N = 512                 # matmul free dim == one PSUM bank (fp32)
N_CHUNKS = NSHARD // CHUNK_N  # 16
TILE_W = 1024                 # psum tile width (2 banks, 2 chunks)
N_PT = NSHARD // TILE_W       # 8 psum tiles per (q-tile, core)
CELLS_PER_TILE = 32           # scan cells per psum tile
N_CELLS = N_PT * CELLS_PER_TILE    # 256 cells per row per core
# Cell c of psum tile m covers candidate columns
#   m*1024 + 16c + [0..15]  and  m*1024 + 512 + 16c + [0..15]
# (32 candidates per cell; the split pairing comes from the GpSimd fold).
TOPC_HALF = 8                 # cells reported per (row, core, cmax-half)
N_HALVES = 2
TOPC = TOPC_HALF * N_HALVES   # 16 cells reported per (row, core)
# Scan statistic: queries are L2-normalized on the host, so s = x_hat . t has
# per-candidate std ~1 and global top-3 values ~3.5+.  Cells are ranked by
# sum(relu(s - RELU_T)) (or relu(max - RELU_T) on the DVE-direct tiles) —
# any cell holding a global top-3 value scores far above typical cells.  The
# top-8 is taken independently over each 128-cell half so a global top-3
# cell would need >= 8 stronger cells in its own half to be lost.
RELU_T = 2.5

_BF16 = mybir.dt.bfloat16
_F32 = mybir.dt.float32
_U32 = mybir.dt.uint32


DIRECT_MOD = 4   # psum tiles with m % DIRECT_MOD == 0 go DVE-direct
PE_ONLY = False  # benchmark probe: skip all PSUM evacuation
FP8_SCALE = 32.0  # pre-scale on normalized queries so fp8 stays in range
# PE weight-load strategy: "self" = every matmul self-loads its stationary
# operand via DoubleRow (K=256/instruction, but the 256-col LDWEIGHTS is
# serialized per matmul); "swi" = DoubleRowSwInterleave with
# host-pre-interleaved x q-tiles (contiguous weight reads); "nodr" = plain
# fp8 matmuls (two K=128 halves accumulated in PSUM) whose contiguous
# 128-col weights qualify for compiler-automatic Fast Weight Load.
LDW_MODE = "self"


def _build(loop_reps=None):
    in_dt = mybir.dt.float8e4
    nc = bacc.Bacc("TRN2", target_bir_lowering=False, debug=False)
    x_shape = [128, 2 * B] if LDW_MODE in ("swi", "selfc") else [F, B]
    t_shape = [128, 2 * NSHARD] if LDW_MODE == "selfc" else [F, NSHARD]
    xT = nc.dram_tensor("xT", x_shape, in_dt, kind="ExternalInput")
    tT = nc.dram_tensor("tT", t_shape, in_dt, kind="ExternalInput")
    out_cm = nc.dram_tensor("cmax_out", [B, N_CELLS], _F32, kind="ExternalOutput")

    with tile.TileContext(nc) as tc, ExitStack() as ctx:
        const = ctx.enter_context(tc.tile_pool(name="const", bufs=1))
        psums = ctx.enter_context(tc.tile_pool(name="ps", bufs=4, space="PSUM"))
        cmaxp = ctx.enter_context(tc.tile_pool(name="cmax", bufs=2))
        stagep = ctx.enter_context(tc.tile_pool(name="stage", bufs=3))
        gpsp = ctx.enter_context(tc.tile_pool(name="gps", bufs=3))

        # Bulk loads: one [128, 2*SIZE] tile per tensor holding both
        # 128-feature halves ((i, col) free layout); chunk operands are
        # strided [p, 2, w] views for DoubleRow.  Two 1MB DMAs stream
        # much faster than 32 64KB ones (shorter prologue).
        x_all = const.tile([128, 2 * B], mybir.dt.float8e4, name="x_all")
        t_all = const.tile([128, 2 * NSHARD], mybir.dt.float8e4,
                           name="t_all")

        if LDW_MODE in ("swi", "selfc"):
            nc.sync.dma_start(x_all[:], xT[:])
        else:
            for f in range(2):
                nc.sync.dma_start(
                    x_all[:, f * B:(f + 1) * B], xT[f * 128:(f + 1) * 128, :]
                )
        if LDW_MODE == "selfc":
            nc.sync.dma_start(t_all[:], tT[:])
        else:
            for f in range(2):
                nc.sync.dma_start(
                    t_all[:, f * NSHARD:(f + 1) * NSHARD],
                    tT[f * 128:(f + 1) * 128, :],
                )

        # [128, 2, w] strided views per q-tile / chunk for DoubleRow, or
        # per-feature-half contiguous 2D views for "nodr".
        t_dr = t_all[:].rearrange("p (i cw) -> p i cw", i=2)
        if LDW_MODE == "swi":
            xT_sb = [x_all[:, q * 2 * Q_TILE:(q + 1) * 2 * Q_TILE]
                     for q in range(N_QT)]
        elif LDW_MODE == "selfc":
            # Host packs each q-tile's / chunk's two K-halves adjacently:
            # x_all col-blocks of 256 per q-tile, t_all col-blocks of 1024
            # per chunk -> DoubleRow operand APs have i-stride 128/512
            # instead of 2048/8192.
            xT_sb = [
                x_all[:, q * 2 * Q_TILE:(q + 1) * 2 * Q_TILE].rearrange(
                    "p (i qw) -> p i qw", i=2
                )
                for q in range(N_QT)
            ]
        elif LDW_MODE == "nodr":
            xT_sb = [
                [x_all[:, f * B + q * Q_TILE:f * B + (q + 1) * Q_TILE]
                 for f in range(2)]
                for q in range(N_QT)
            ]
        else:
            x_dr = x_all[:].rearrange("p (i qw) -> p i qw", i=2)
            xT_sb = [
                x_dr[:, :, q * Q_TILE:(q + 1) * Q_TILE] for q in range(N_QT)
            ]
        if LDW_MODE == "nodr":
            tT_sb = [
                [t_all[:, f * NSHARD + c * CHUNK_N:f * NSHARD + (c + 1) * CHUNK_N]
                 for f in range(2)]
                for c in range(N_CHUNKS)
            ]
        elif LDW_MODE == "selfc":
            tT_sb = [
                t_all[:, c * 2 * CHUNK_N:(c + 1) * 2 * CHUNK_N].rearrange(
                    "p (i cw) -> p i cw", i=2
                )
                for c in range(N_CHUNKS)
            ]
        else:
            tT_sb = [
                t_dr[:, :, c * CHUNK_N:(c + 1) * CHUNK_N] for c in range(N_CHUNKS)
            ]

        neg_t = const.tile([128, 1], _F32, name="neg_t")
        nc.vector.memset(neg_t[:], -RELU_T * FP8_SCALE)

        CPW = TILE_W // CHUNK_N  # chunks per psum tile
        TILES_PER_WAVE = 2       # tiles per accumulation wave
        WAVES = N_PT // TILES_PER_WAVE  # 4 waves per q-tile

        def compute():
            _compute(nc, tc, xT_sb, tT_sb, neg_t, cmaxp, psums, stagep,
                     gpsp, out_cm, CPW, TILES_PER_WAVE, WAVES)

        if loop_reps is not None:
            with tc.For_i(0, loop_reps, 1):
                compute()
        else:
            compute()
    nc.compile()
    return nc


def _compute(nc, tc, xT_sb, tT_sb, neg_t, cmaxp, psums, stagep, gpsp,
             out_cm, CPW, TILES_PER_WAVE, WAVES):
        pmode = (
            mybir.MatmulPerfMode.DoubleRowSwInterleave
            if LDW_MODE == "swi"
            else mybir.MatmulPerfMode.DoubleRow
        )

        def emit_wave(q, w, cmax):
                pss = [
                    psums.tile([128, TILE_W], _F32, tag="ps", name=f"ps_{q}_{w}_{j}")
                    for j in range(TILES_PER_WAVE)
                ]
                if LDW_MODE == "nodr":
                    # Two K=128 matmuls accumulate each chunk; both operands
                    # are contiguous 2D APs so the 128-col weight load takes
                    # the compiler's Fast-Weight-Load path.
                    for f in range(2):
                        for j in range(TILES_PER_WAVE):
                            for hh in range(CPW):
                                c = (w * TILES_PER_WAVE + j) * CPW + hh
                                nc.tensor.matmul(
                                    pss[j][:, hh * CHUNK_N:(hh + 1) * CHUNK_N],
                                    xT_sb[q][f],
                                    tT_sb[c][f],
                                    start=(f == 0),
                                    stop=(f == 1),
                                )
                else:
                    for j in range(TILES_PER_WAVE):
                        for hh in range(CPW):
                            c = (w * TILES_PER_WAVE + j) * CPW + hh
                            nc.tensor.matmul(
                                pss[j][:, hh * CHUNK_N:(hh + 1) * CHUNK_N],
                                xT_sb[q],
                                tT_sb[c],
                                start=True,
                                stop=True,
                                perf_mode=pmode,
                            )
                for j in range(TILES_PER_WAVE):
                    m = w * TILES_PER_WAVE + j
                    cm_out = cmax[:, m * CELLS_PER_TILE:(m + 1) * CELLS_PER_TILE]
                    if PE_ONLY:
                        if m == 0:
                            nc.vector.memset(cmax[:], 0.0)
                        continue
                    if DIRECT_MOD > 0 and m % DIRECT_MOD == 0:
                        # DVE evacuates+reduces this tile straight from PSUM
                        # (max over 32 contiguous candidates per cell).
                        # Raw max out; the host subtracts RELU_T from these
                        # columns before selection (saves an ACT op here).
                        nc.vector.tensor_reduce(
                            out=cm_out,
                            in_=pss[j][:].rearrange("p (c e) -> p c e", e=32),
                            axis=mybir.AxisListType.X,
                            op=mybir.AluOpType.max,
                        )
                    else:
                        # Offload evacuation: ScalarE applies relu(s - T) on
                        # the way PSUM->SBUF, GpSimd ADD-folds the two
                        # 512-halves, DVE sum-reduces 16-wide cells.
                        st = stagep.tile([128, TILE_W], _BF16, tag="st",
                                         name=f"st_{q}_{m}")
                        nc.scalar.activation(
                            st[:], pss[j][:],
                            mybir.ActivationFunctionType.Relu,
                            bias=neg_t[:],
                        )
                        gp = gpsp.tile([128, TILE_W // 2], _BF16, tag="gp",
                                       name=f"gp_{q}_{m}")
                        nc.gpsimd.tensor_add(
                            gp[:], st[:, 0:TILE_W // 2],
                            st[:, TILE_W // 2:TILE_W]
                        )
                        nc.vector.tensor_reduce(
                            out=cm_out,
                            in_=gp[:].rearrange("p (c e) -> p c e", e=16),
                            axis=mybir.AxisListType.X,
                            op=mybir.AluOpType.add,
                        )
        for q in range(N_QT):
            cmax = cmaxp.tile([128, N_CELLS], _F32, name=f"cmax_{q}")
            for w in range(WAVES):
                emit_wave(q, w, cmax)
            qs = slice(q * Q_TILE, (q + 1) * Q_TILE)
            nc.sync.dma_start(out_cm[qs, :], cmax[:])


def _cells_to_blocks(cid):
    """Map per-(core,row) cell ids -> the two 16-row train blocks each covers.

    cid: int64 array of cell ids in [0, N_CELLS), AFTER the half-offset fix.
    Returns (blk0, blk1) within-shard block indices (block = 16 train rows).
    DVE-direct tiles (m % DIRECT_MOD == 0) use 32 contiguous candidates;
    offloaded tiles pair candidates {16c, 16c+512} (the GpSimd fold).
    """
    m = cid // CELLS_PER_TILE
    c = cid % CELLS_PER_TILE
    direct = (m % DIRECT_MOD == 0) if DIRECT_MOD > 0 else np.zeros_like(m, bool)
    blk0 = m * (TILE_W // 16) + np.where(direct, 2 * c, c)
    blk1 = blk0 + np.where(direct, 1, TILE_W // 32)
    return blk0, blk1


def _host_adjust(cm):
    """Direct tiles report raw cell max; put them on the relu(.-T) scale."""
    shift = RELU_T * FP8_SCALE
    m = np.arange(N_CELLS) // CELLS_PER_TILE
    direct = (m % DIRECT_MOD == 0) if DIRECT_MOD > 0 else np.zeros_like(m, bool)
    cm[..., direct] -= shift
    return cm


_CACHE = {}


def _prep_inputs(x_input, train_inputs):
    x = np.asarray(x_input, np.float32)
    # Row-normalize queries so the global RELU_T threshold is calibrated.
    xh = x / (np.linalg.norm(x, axis=1, keepdims=True) + 1e-30)
    xh = xh * FP8_SCALE
    in_np_dt = ml_dtypes.float8_e4m3
    xT = np.ascontiguousarray(xh.T).astype(in_np_dt)
    if LDW_MODE == "swi":
        # Per q-tile: [A127 B127 A126 B126 ... A0 B0] per partition, where
        # A/B are the two 128-feature halves and columns are reversed.
        xa = xT[:128].reshape(128, N_QT, Q_TILE)[:, :, ::-1]
        xb = xT[128:].reshape(128, N_QT, Q_TILE)[:, :, ::-1]
        xT = np.ascontiguousarray(
            np.stack([xa, xb], axis=-1).reshape(128, 2 * B)
        )
    elif LDW_MODE == "selfc":
        # Per q-tile: [A-half cols | B-half cols] adjacent 256-col blocks.
        xa = xT[:128].reshape(128, N_QT, Q_TILE)
        xb = xT[128:].reshape(128, N_QT, Q_TILE)
        xT = np.ascontiguousarray(
            np.stack([xa, xb], axis=2).reshape(128, 2 * B)
        )
    in_maps = []
    for s in range(N_CORES):
        shard = np.asarray(train_inputs[s * NSHARD:(s + 1) * NSHARD], np.float32)
        tTs = np.ascontiguousarray(shard.T).astype(in_np_dt)
        if LDW_MODE == "selfc":
            ta = tTs[:128].reshape(128, N_CHUNKS, CHUNK_N)
            tb = tTs[128:].reshape(128, N_CHUNKS, CHUNK_N)
            tTs = np.ascontiguousarray(
                np.stack([ta, tb], axis=2).reshape(128, 2 * NSHARD)
            )
        in_maps.append({"xT": xT, "tT": tTs})
    return in_maps


def _run_device(x_input, train_inputs, trace=False, **kw):
    if "nc" not in _CACHE:
        _CACHE["nc"] = _build()
    nc = _CACHE["nc"]
    in_maps = _prep_inputs(x_input, train_inputs)
    return run_bass_kernel_spmd(
        nc, in_maps, core_ids=list(range(N_CORES)), trace=trace, **kw
    )


def kernel(x_input, train_inputs, features, train_labels, num_k, num_labels):
    x = np.asarray(x_input, dtype=np.float32)
    train = np.asarray(train_inputs, dtype=np.float32)
    feats = np.asarray(features, dtype=np.float32)
    labels = np.asarray(train_labels)
    k = int(num_k)
    L = int(num_labels)

    res = _run_device(x, train)
    cm = np.stack(
        [np.asarray(res.results[s]["cmax_out"]) for s in range(N_CORES)], axis=0
    )  # [cores, B, N_CELLS] f32 cell statistics
    cm = _host_adjust(cm)

    # Host-side selection: top-TOPC cells per (core, row) by statistic.
    flat = cm.reshape(-1, N_CELLS)
    part = np.argpartition(-flat, TOPC - 1, axis=1)[:, :TOPC]
    cid = part.reshape(N_CORES, B, TOPC).astype(np.int64)

    # Expand top cells to candidate BLOCKS of 16 contiguous train rows.
    blk0, blk1 = _cells_to_blocks(cid)
    blk = np.stack([blk0, blk1], axis=-1)             # [cores, B, TOPC, 2]
    blk = blk + (np.arange(N_CORES, dtype=np.int64) * (NSHARD // 16))[
        :, None, None, None
    ]
    blk = blk.transpose(1, 0, 2, 3).reshape(B, -1)    # [B, cores*TOPC*2=256]
    blk = np.sort(blk, axis=1)
    NBLK = blk.shape[1]
    dupb = np.zeros(blk.shape, dtype=bool)
    dupb[:, 1:] = blk[:, 1:] == blk[:, :-1]

    # Refinement: coarse f32 pass narrows ~4k candidates/row to 8, then an
    # exact float64 pass ranks those with the reference's tie-breaking.
    w = feats[None, :] * train
    right32 = np.einsum("nf,nf->n", w, w, dtype=np.float32)
    left32 = np.einsum("bf,bf->b", x, x, dtype=np.float32)
    w64 = w.astype(np.float64)
    x64 = x.astype(np.float64)
    left64 = np.einsum("bf,bf->b", x64, x64)

    train_blocks = train.reshape(N_TRAIN // 16, 16 * F)
    NARROW = 8
    topk_idx = np.empty((B, k), dtype=np.int64)
    CH = 128
    gbuf = np.empty((CH * NBLK, 16 * F), dtype=np.float32)
    for r0 in range(0, B, CH):
        r1 = min(B, r0 + CH)
        bi = blk[r0:r1]                                # [rows, NBLK]
        ci = (bi[:, :, None] * 16 + np.arange(16)).reshape(r1 - r0, -1)
        np.take(train_blocks, bi.ravel(), axis=0, out=gbuf)
        tcand = gbuf.reshape(r1 - r0, NBLK * 16, F)    # [rows, nc, F]
        cross = np.matmul(tcand, x[r0:r1][:, :, None])[..., 0]
        d32 = np.sqrt(left32[r0:r1, None] + right32[ci]) - 2.0 * cross
        d32.reshape(r1 - r0, NBLK, 16)[dupb[r0:r1]] = np.inf
        part = np.argpartition(d32, NARROW, axis=1)[:, :NARROW]
        ci8 = np.take_along_axis(ci, part, axis=1)     # [rows, 8] distinct
        ci8.sort(axis=1)
        # exact f64 distances for the 8 finalists
        t8 = train[ci8].astype(np.float64)
        cross8 = np.matmul(t8, x64[r0:r1][:, :, None])[..., 0]
        w8 = w64[ci8]
        r8 = np.einsum("bkf,bkf->bk", w8, w8)
        d8 = np.sqrt(left64[r0:r1, None] + r8) - 2.0 * cross8
        dup8 = np.zeros(ci8.shape, dtype=bool)
        dup8[:, 1:] = ci8[:, 1:] == ci8[:, :-1]
        d8[dup8] = np.inf
        order = np.argsort(d8, axis=1, kind="stable")[:, :k]
        topk_idx[r0:r1] = np.take_along_axis(ci8, order, axis=1)

    lab = labels[topk_idx]               # [B, k] (int64)
    lab_kb = lab.reshape(k, B)           # faithful [B,k] -> [k,B] reshape
    outputs = lab_kb.sum(axis=0) // k
    out = np.zeros((B, L), dtype=np.float32)
    out[np.arange(B), outputs] = 1.0
    return out

